# revision 2
# baseline (speedup 1.0000x reference)
"""Multi-head attention forward for Trainium2, 8 NeuronCores — v2.

Problem: B=4, S=2048, D=1024, H=16 heads (dk=64), fp32 reference:
  q/k/v = x @ W{q,k,v}^T + b ; heads split; softmax(q k^T / 8) v ; out @ Wo^T + bo

Sharding: 8 cores = 4 batches x 2 head-groups (8 heads each), Megatron-style:
each core computes its batch's attention for its 8 heads plus the partial
output projection (Wo column slice); host sums the two partials per batch.

v2 design (vs the v1 baseline):
  - softmax denominator folded into the PV matmuls: each head's stationary is
    an augmented [128,65] [V|1] slice, so PSUM row 64 accumulates the den for
    free. The old ones32 denominator matmuls (~25% of phase-B PE) are gone,
    and normalization is one recip + one gpsimd partition_broadcast + two
    scaled copies (h1's realigned 0:64 -> 64:128 by a small SBUF DMA).
  - one [128,1024] exp per key-tile (both heads side by side) instead of two.
  - score PSUM double-buffered + QK emitted one k-tile ahead of PV so the PE
    never head-of-line blocks on the activation engine.
  - phase A (next pair's Q/K projection) and phase C (output projection)
    chains interleaved into phase B's instruction stream to fill PE slack.
  - all matmul inputs fp16 (halves input DMA; 1 cyc/row); fp32 PSUM accum.
  - x/Wv staged per contraction-chunk in separate tiles so compute starts as
    soon as the first chunk lands; first half of V runs ec-major across an
    8-bank PSUM scope to overlap the input DMA.
  - V/output biases added on the DVE during PSUM drain via gpsimd
    partition_broadcast bias tiles (no ones-row bias matmuls).
"""

import sys

sys.path.insert(0, "/opt/trn_rl_repo")

import numpy as np

import concourse.bass as bass  # noqa: F401
import concourse.mybir as mybir
import concourse.tile as tile
from concourse import bacc, bass_utils  # noqa: F401

B, S, D, H = 4, 2048, 1024, 16
DK = D // H          # 64
G = 2                # head groups (tensor-parallel factor)
DL = D // G          # 512 local features per core
NPAIR = DL // 128    # 4 head-pairs per core
EC = D // 128        # 8 contraction chunks for projections
ST = S // 128        # 16 s-tiles
KT = S // 128        # 16 key tiles
NQH = 4              # 512-query chunks per pair

F32 = mybir.dt.float32
F16 = mybir.dt.float16

_CACHED = {}


def _build_nc(loop_n=1):
    nc = bacc.Bacc(None, target_bir_lowering=False)

    xT = nc.dram_tensor("xT", [D, S], F16, kind="ExternalInput")
    wqT = nc.dram_tensor("wqT", [D, DL], F16, kind="ExternalInput")
    wkT = nc.dram_tensor("wkT", [D, DL], F16, kind="ExternalInput")
    wvT = nc.dram_tensor("wvT", [D, DL], F16, kind="ExternalInput")
    woT = nc.dram_tensor("woT", [DL, D], F16, kind="ExternalInput")
    bq = nc.dram_tensor("bq", [DL], F32, kind="ExternalInput")
    bk = nc.dram_tensor("bk", [DL], F32, kind="ExternalInput")
    bv = nc.dram_tensor("bv", [1, DL], F16, kind="ExternalInput")
    bo = nc.dram_tensor("bo", [1, D], F32, kind="ExternalInput")
    y = nc.dram_tensor("y", [S, D], F32, kind="ExternalOutput")

    EXP = mybir.ActivationFunctionType.Exp

    with tile.TileContext(nc) as tc:
      for _rep in range(loop_n):
        with (
            tc.tile_pool(name="main", bufs=1) as pmain,
            tc.tile_pool(name="qkt", bufs=2) as pqkt,
            tc.tile_pool(name="ptile", bufs=3) as ppt,
            tc.tile_pool(name="osb", bufs=2) as posb,
            tc.tile_pool(name="rcp", bufs=2) as prcp,
            tc.tile_pool(name="rep", bufs=2) as prep,
            tc.tile_pool(name="shf", bufs=2) as pshf,
            tc.tile_pool(name="ytile", bufs=3) as pyt,
        ):
            # ---- persistent tiles
            # per (s-tile, pair): 130 cols = [V0(64)|1|V1(64)|1]: each head's
            # PV stationary is a [128,65] slice whose trailing ones column
            # emits the softmax denominator into PSUM output row 64
            vt = pmain.tile([128, ST, NPAIR, 130], F16, tag="vt")
            ot = pmain.tile([128, NPAIR, S], F16, tag="ot")
            bqt = pmain.tile([128, NPAIR], F32, tag="bqt")
            bkt = pmain.tile([128, NPAIR], F32, tag="bkt")
            bvt = pmain.tile([1, DL], F16, tag="bvt")
            bot = pmain.tile([1, D], F32, tag="bot")
            bvr = pmain.tile([128, NPAIR, 128], F16, tag="bvr")
            bor = pmain.tile([128, D], F32, tag="bor")
            wot = pmain.tile([128, NPAIR, D], F16, tag="wot")

            nc.vector.memset(vt[:, :, :, 64:65], 1.0)
            nc.vector.memset(vt[:, :, :, 129:130], 1.0)

            def issue_bias_dmas():
                nc.sync.dma_start(bqt[:],
                                  bq.ap().rearrange("(p d) -> d p", d=128))
                nc.sync.dma_start(bkt[:],
                                  bk.ap().rearrange("(p d) -> d p", d=128))
                nc.sync.dma_start(bvt[:], bv.ap())
                nc.sync.dma_start(bot[:], bo.ap())
                nc.gpsimd.partition_broadcast(bvr[:], bvt[:])
                nc.gpsimd.partition_broadcast(bor[:], bot[:])

            with (
                tc.tile_pool(name="xw", bufs=1) as pxw,
                tc.tile_pool(name="wqk", bufs=2) as pwqk,
            ):
                xts = [pxw.tile([128, S], F16, tag=f"xt{ec}", name=f"xt{ec}")
                       for ec in range(EC)]
                wvts = [pxw.tile([128, DL], F16, tag=f"wvt{ec}", name=f"wvt{ec}")
                        for ec in range(EC)]

                wq_tiles = [None] * NPAIR
                wk_tiles = [None] * NPAIR
                pwqk_holder = []

                def fetch_wqk(pool, p):
                    wqp = pool.tile([128, EC, 128], F16, tag="wqp", name=f"wqp{p}")
                    wkp = pool.tile([128, EC, 128], F16, tag="wkp", name=f"wkp{p}")
                    cs = slice(p * 128, (p + 1) * 128)
                    nc.sync.dma_start(
                        wqp[:], wqT.ap()[:, cs].rearrange("(e p) c -> p e c", p=128))
                    nc.sync.dma_start(
                        wkp[:], wkT.ap()[:, cs].rearrange("(e p) c -> p e c", p=128))
                    wq_tiles[p], wk_tiles[p] = wqp, wkp

                def drain_v(st, src):
                    # src: [128, NPAIR, 128] f32 PSUM view for s-tile st
                    nc.vector.tensor_add(
                        vt[:, st, :, 0:64], src[:, :, 0:64], bvr[:, :, 0:64])
                    nc.vector.tensor_add(
                        vt[:, st, :, 65:129], src[:, :, 64:128], bvr[:, :, 64:128])

                # x/Wv chunks first: the SP sequencer issues DMAs at
                # ~565ns each, so anything queued ahead delays V's start
                for ec in range(EC):
                    nc.sync.dma_start(
                        xts[ec][:], xT.ap()[ec * 128:(ec + 1) * 128, :])
                    nc.sync.dma_start(
                        wvts[ec][:], wvT.ap()[ec * 128:(ec + 1) * 128, :])
                fetch_wqk(pwqk, 0)
                issue_bias_dmas()
                for dc in range(NPAIR):
                    nc.sync.dma_start(
                        wot[:, dc], woT.ap()[dc * 128:(dc + 1) * 128, :])

                # ---- V first half, ec-major over an 8-bank PSUM scope:
                # compute advances chunk-by-chunk as the x DMA lands
                with tc.tile_pool(name="psV", bufs=1, space="PSUM") as psV:
                    vhps = psV.tile([128, 8, NPAIR, 128], F32, tag="vhps")
                    for ec in range(EC):
                        for st in range(8):
                            nc.tensor.matmul(
                                vhps[:, st],
                                xts[ec][:, st * 128:(st + 1) * 128],
                                wvts[ec][:],
                                start=(ec == 0), stop=(ec == EC - 1))
                            if ec == EC - 1:
                                # drain immediately so the adds pipeline with
                                # the remaining last-chunk matmuls
                                drain_v(st, vhps[:, st])

                with (
                    tc.tile_pool(name="psS", bufs=2, space="PSUM") as psS,
                    tc.tile_pool(name="psO", bufs=1, space="PSUM") as psO,
                    tc.tile_pool(name="psA", bufs=2, space="PSUM") as psA,
                ):
                    def phase_v_tail():
                        for st in range(8, ST):
                            vps = psA.tile([128, NPAIR, 128], F32, tag="psA",
                                           name="vps")
                            for ec in range(EC):
                                nc.tensor.matmul(
                                    vps[:],
                                    xts[ec][:, st * 128:(st + 1) * 128],
                                    wvts[ec][:],
                                    start=(ec == 0), stop=(ec == EC - 1))
                            drain_v(st, vps)

                    qts = [None] * NPAIR
                    kts = [None] * NPAIR

                    def phase_a_chains(p):
                        """8 closures, each one 512-col Q^T/K^T chain."""
                        wqp, wkp = wq_tiles[p], wk_tiles[p]
                        qt = pqkt.tile([128, S], F16, tag="qt", name=f"qt{p}")
                        kt = pqkt.tile([128, S], F16, tag="kt", name=f"kt{p}")
                        qts[p], kts[p] = qt, kt

                        def chain(dst, wp, bias, qc):
                            def run():
                                ps = psA.tile([128, 512], F32, tag="psA",
                                              name="aps")
                                for ec in range(EC):
                                    nc.tensor.matmul(
                                        ps[:], wp[:, ec],
                                        xts[ec][:, qc * 512:(qc + 1) * 512],
                                        start=(ec == 0), stop=(ec == EC - 1))
                                nc.vector.tensor_scalar_add(
                                    dst[:, qc * 512:(qc + 1) * 512], ps[:],
                                    bias[:, p:p + 1])
                            return run

                        # K chunks first (phase B needs all of K up front)
                        return ([chain(kt, wkp, bkt, qc) for qc in range(4)]
                                + [chain(qt, wqp, bqt, qc) for qc in range(4)])

                    def phase_c_chunks(qh, pool, tag):
                        """Output projection for one 512-query block."""
                        chunks = []
                        for sti in range(4):
                            st = qh * 4 + sti
                            ss = slice(st * 128, (st + 1) * 128)
                            for e2 in range(2):
                                es = slice(e2 * 512, (e2 + 1) * 512)

                                def run(ss=ss, es=es):
                                    if tag == "stt":
                                        yfull = pool.tile([128, 1024], F32,
                                                          tag=tag, name="ypsS")
                                        yps = yfull[:, 0:512]
                                    else:
                                        yps = pool.tile([128, 512], F32,
                                                        tag=tag, name="ypsA")[:]
                                    for dc in range(NPAIR):
                                        nc.tensor.matmul(
                                            yps, ot[:, dc, ss], wot[:, dc, es],
                                            start=(dc == 0),
                                            stop=(dc == NPAIR - 1))
                                    yt = pyt.tile([128, 512], F32, tag="yt")
                                    nc.vector.tensor_add(yt[:], yps, bor[:, es])
                                    nc.sync.dma_start(y.ap()[ss, es], yt[:])
                                chunks.append(run)
                        return chunks

                    def phase_b(p, feeders, feed_every, post_qh=None):
                        """Attention for head-pair p. Pops one feeder closure
                        every feed_every k-tile units to fill PE slack."""
                        qt, kt = qts[p], kts[p]
                        for qh in range(NQH):
                            q0 = qh * 512
                            qs = slice(q0, q0 + 512)
                            oth = psO.tile([65, 1024], F32, tag="oth")
                            pts = []

                            def qk(kti):
                                ks = slice(kti * 128, (kti + 1) * 128)
                                st_t = psS.tile([128, 1024], F32, tag="stt",
                                                name="stt")
                                nc.tensor.matmul(
                                    st_t[:, 0:512], kt[0:64, ks], qt[0:64, qs],
                                    start=True, stop=True, tile_position=(0, 0))
                                nc.tensor.matmul(
                                    st_t[:, 512:1024], kt[64:128, ks],
                                    qt[64:128, qs],
                                    start=True, stop=True, tile_position=(64, 0))
                                p_t = ppt.tile([128, 1024], F16, tag="pt")
                                nc.scalar.activation(p_t[:], st_t[:], EXP,
                                                     scale=0.125)
                                pts.append(p_t)

                            def pv(kti):
                                first, last = kti == 0, kti == KT - 1
                                p_t = pts[kti]
                                # each head: [V|1] -> rows 0:65, den at row 64
                                nc.tensor.matmul(
                                    oth[0:65, 0:512], vt[:, kti, p, 0:65],
                                    p_t[:, 0:512], start=first, stop=last)
                                nc.tensor.matmul(
                                    oth[0:65, 512:1024], vt[:, kti, p, 65:130],
                                    p_t[:, 512:1024], start=first, stop=last)

                            qk(0)
                            for kti in range(1, KT):
                                qk(kti)
                                # at block start, feed BEFORE pv(0): the chain
                                # covers the previous block's oth drain; later
                                # slots feed after pv so exp stays just-in-time
                                feed = (kti % feed_every == 1 and feeders)
                                if feed and kti == 1:
                                    feeders.pop(0)()
                                pv(kti - 1)
                                if feed and kti != 1:
                                    feeders.pop(0)()
                            pv(KT - 1)

                            # normalization tail: DVE drain frees oth fast;
                            # one recip over both dens; one broadcast; h1's
                            # normalized block realigned 0:64 -> 64:128 by DMA
                            if p == NPAIR - 1 and qh == NQH - 1:
                                # nothing reuses oth after the final block:
                                # skip the drain copy and read PSUM directly
                                oth_sb = oth
                            else:
                                oth_sb = posb.tile([65, 1024], F32, tag="osb")
                                nc.vector.tensor_copy(oth_sb[:], oth[:])
                            r1 = prcp.tile([65, 1024], F32, tag="r1")
                            nc.vector.reciprocal(r1[64:65, :], oth_sb[64:65, :])
                            # HW partition_broadcast reads physical partition
                            # 0 regardless of AP base: hop the recip row down
                            # with a tiny SBUF DMA first
                            r0 = prcp.tile([1, 1024], F32, tag="r0")
                            nc.sync.dma_start(r0[:], r1[64:65, :])
                            rt = prep.tile([64, 1024], F32, tag="rt")
                            nc.gpsimd.partition_broadcast(rt[:], r0[:])
                            nc.vector.tensor_mul(
                                ot[0:64, p, qs], oth_sb[0:64, 0:512],
                                rt[:, 0:512])
                            o1n = pshf.tile([64, 512], F16, tag="o1n")
                            nc.vector.tensor_mul(
                                o1n[:], oth_sb[0:64, 512:1024], rt[:, 512:1024])
                            nc.sync.dma_start(ot[64:128, p, qs], o1n[:])
                            if post_qh is not None:
                                post_qh(qh)

                    phase_v_tail()
                    for ch in phase_a_chains(0):
                        ch()
                    for p in range(NPAIR):
                        if p + 1 < NPAIR:
                            fetch_wqk(pwqk, p + 1)
                            phase_b(p, phase_a_chains(p + 1), feed_every=8)
                        else:
                            c_pending = []

                            def post_qh(qh):
                                if qh < NQH - 1:
                                    c_pending.extend(
                                        phase_c_chunks(qh, psA, "psA"))
                                else:
                                    # flush block: rotate yps through the
                                    # now-idle score banks too for a deeper
                                    # PSUM pipeline
                                    ca = phase_c_chunks(qh, psA, "psA")
                                    cs_ = phase_c_chunks(qh, psS, "stt")
                                    c_pending.extend(
                                        (cs_ if i % 2 else ca)[i]
                                        for i in range(8))

                            phase_b(p, c_pending, feed_every=2,
                                    post_qh=post_qh)
                            for ch in c_pending:
                                ch()

    nc.compile()
    return nc


def _get_nc(loop_n=1):
    key = f"nc{loop_n}"
    if key not in _CACHED:
        _CACHED[key] = _build_nc(loop_n)
    return _CACHED[key]


def make_in_maps(encoder_input, Wq_w, Wq_b, Wk_w, Wk_b, Wv_w, Wv_b, Wo_w, Wo_b):
    """Per-core input dict list (host-side sharding + dtype prep)."""
    woT_f16 = {}
    in_maps = []
    for core in range(8):
        b, g = divmod(core, G)
        gs = slice(g * DL, (g + 1) * DL)
        if g not in woT_f16:
            woT_f16[g] = np.ascontiguousarray(Wo_w[:, gs].T).astype(np.float16)
        in_maps.append({
            "xT": np.ascontiguousarray(encoder_input[b].T).astype(np.float16),
            "wqT": np.ascontiguousarray(Wq_w[gs, :].T).astype(np.float16),
            "wkT": np.ascontiguousarray(Wk_w[gs, :].T).astype(np.float16),
            "wvT": np.ascontiguousarray(Wv_w[gs, :].T).astype(np.float16),
            "woT": woT_f16[g],
            "bq": np.ascontiguousarray(Wq_b[gs]).astype(np.float32),
            "bk": np.ascontiguousarray(Wk_b[gs]).astype(np.float32),
            "bv": Wv_b[gs].astype(np.float16).reshape(1, DL),
            "bo": (Wo_b if g == 0 else np.zeros_like(Wo_b))
                  .astype(np.float32).reshape(1, D),
        })
    return in_maps


def _get_runner():
    """Build the 8-core SPMD executable once and cache it, so repeated
    kernel() calls skip jax re-tracing and NEFF compilation."""
    if "runner" in _CACHED:
        return _CACHED["runner"]

    import jax
    from jax.sharding import Mesh, NamedSharding, PartitionSpec
    from jax.experimental.shard_map import shard_map
    from concourse import bass2jax
    from concourse.bass2jax import _bass_exec_p, install_neuronx_cc_hook

    nc = _get_nc()
    install_neuronx_cc_hook()
    partition_name = nc.partition_id_tensor.name if nc.partition_id_tensor else None
    in_names, out_names, out_avals, zero_outs = [], [], [], []
    for alloc in nc.m.functions[0].allocations:
        if not isinstance(alloc, mybir.MemoryLocationSet):
            continue
        name = alloc.memorylocations[0].name
        if alloc.kind == "ExternalInput":
            if name != partition_name:
                in_names.append(name)
        elif alloc.kind == "ExternalOutput":
            out_names.append(name)
            shape = tuple(alloc.tensor_shape)
            dtype = mybir.dt.np(alloc.dtype)
            out_avals.append(jax.core.ShapedArray(shape, dtype))
            zero_outs.append(np.zeros(shape, dtype))
    n_params, n_outs = len(in_names), len(out_avals)
    all_names = in_names + out_names + ([partition_name] if partition_name else [])

    def _body(*args):
        operands = list(args)
        if partition_name is not None:
            operands.append(bass2jax.partition_id_tensor())
        outs = _bass_exec_p.bind(
            *operands,
            out_avals=tuple(out_avals),
            in_names=tuple(all_names),
            out_names=tuple(out_names),
            lowering_input_output_aliases=(),
            sim_require_finite=True,
            sim_require_nnan=True,
            nc=nc,
        )
        return tuple(outs)

    devices = jax.devices()[:8]
    mesh = Mesh(np.asarray(devices), ("core",))
    f = jax.jit(
        shard_map(
            _body, mesh=mesh,
            in_specs=(PartitionSpec("core"),) * (n_params + n_outs),
            out_specs=(PartitionSpec("core"),) * n_outs,
            check_rep=False,
        ),
        donate_argnums=tuple(range(n_params, n_params + n_outs)),
        keep_unused=True,
    )
    shard = NamedSharding(mesh, PartitionSpec("core"))
    state = {
        "f": f, "in_names": in_names, "out_names": out_names,
        "zero_outs": zero_outs, "shard": shard, "jax": jax, "last_outs": None,
    }
    _CACHED["runner"] = state
    return state


def kernel(encoder_input, attention_mask, Wq_w, Wq_b, Wk_w, Wk_b, Wv_w, Wv_b,
           Wo_w, Wo_b):
    del attention_mask  # dead input in the reference forward
    encoder_input = np.asarray(encoder_input, dtype=np.float32)
    Wq_w = np.asarray(Wq_w, dtype=np.float32)
    Wk_w = np.asarray(Wk_w, dtype=np.float32)
    Wv_w = np.asarray(Wv_w, dtype=np.float32)
    Wo_w = np.asarray(Wo_w, dtype=np.float32)
    Wq_b = np.asarray(Wq_b, dtype=np.float32)
    Wk_b = np.asarray(Wk_b, dtype=np.float32)
    Wv_b = np.asarray(Wv_b, dtype=np.float32)
    Wo_b = np.asarray(Wo_b, dtype=np.float32)

    r = _get_runner()
    jax = r["jax"]

    in_maps = make_in_maps(encoder_input, Wq_w, Wq_b, Wk_w, Wk_b,
                           Wv_w, Wv_b, Wo_w, Wo_b)

    concat_in = [
        jax.device_put(
            np.concatenate([in_maps[c][n] for c in range(8)], axis=0), r["shard"])
        for n in r["in_names"]
    ]
    outs = r["last_outs"]
    if outs is None:
        outs = [
            jax.device_put(
                np.zeros((8 * z.shape[0], *z.shape[1:]), z.dtype), r["shard"])
            for z in r["zero_outs"]
        ]
    outs = r["f"](*concat_in, *outs)
    np_outs = [np.asarray(o) for o in outs]
    # keep the returned device buffers to donate on the next call
    r["last_outs"] = list(outs)

    per_core = {}
    for i, nme in enumerate(r["out_names"]):
        full = np_outs[i].reshape(8, -1, *np_outs[i].shape[1:])
        per_core[nme] = full

    y = per_core["y"]
    out = np.empty((B, S, D), dtype=np.float32)
    for b in range(B):
        out[b] = y[G * b] + y[G * b + 1]
    return out
